# revision 4
# baseline (speedup 1.0000x reference)
"""DLRM pairwise-interaction kernel v2 for Trainium2 (8 NeuronCores).

Per batch b: Z_b = X_b @ X_b^T (X_b is [64, 256]); emit strict lower triangle
row-major -> [B, 2016] fp32. Pure data parallel over B (1024 batches/core).

v2 pipeline (vs baseline):
  - Input loads are SWDGE cast-DMAs (fp32 HBM -> fp16 SBUF): the f32->f16
    cast costs zero compute-engine time, and with no fp32 SBUF->SBUF copies
    DVE never enters a 2-port perf mode, so GpSimd / SWDGE descriptor
    generation is never port-starved.
  - GpSimd's queue is mostly cast-loads; its triangle-compaction share is
    emitted two half-supertiles late so loads are never head-of-line blocked.
  - Matmuls use full-width 128-column fp16 weights (a whole 2-batch pair) so
    FWL engages: 2 accumulating K=128 matmuls per pair instead of 4
    LDWEIGHTS-bound 64-column matmuls.  The off-diagonal cross-batch junk in
    each [128,128] PSUM tile is dropped during a selective Z evacuation
    (two half-partition copies -> clean z_sb), which costs the same engine
    time as one full-width copy.
  - Work is pipelined at half-supertile (64 pairs) granularity; stores are
    HWDGE (sync) 2 MB per 128-pair supertile.
"""
import sys
import numpy as np

sys.path.insert(0, "/opt/trn_rl_repo")

import concourse.bass as bass
import concourse.mybir as mybir
import concourse.tile as tile
from concourse.vector_clock import ScopedClock

F32 = mybir.dt.float32
F16 = mybir.dt.float16

B, N, D, TRI = 8192, 64, 256, 2016
NCORES = 8
NB = B // NCORES           # batches per core
ST_PAIRS = 128             # pairs per supertile (=256 batches, one store)
HALF_PAIRS = 64            # z/stage-C granularity
CHUNK_PAIRS = 8            # pairs per cast-load DMA (1 MB read / 512 KB write)

# ---------------------------------------------------------------------------
# Workaround for walrus builds that only accept ONE sync-wait per instruction.
# ---------------------------------------------------------------------------
_orig_commit = tile.TileContext._commit_instruction


def _split_waits(self, inst):
    si = getattr(inst, "sync_info", None)
    if si is None or not si.on_wait or len(si.on_wait) <= 1:
        return
    if inst.engine == mybir.EngineType.Unassigned:
        return
    waits = list(si.on_wait)
    inst.sync_info = mybir.SyncInfo(on_wait=[waits[-1]], on_update=list(si.on_update))
    for w in waits[:-1]:
        nop = mybir.InstNoOp(name=f"{inst.name}-wsplit-{w.id}", ins=[], outs=[])
        nop.engine = inst.engine
        nop.sync_info = mybir.SyncInfo(on_wait=[w], on_update=[])
        _orig_commit(self, nop, lazy_reg_writes=False)


def _commit_instruction_split(self, inst, lazy_reg_writes=True):
    _split_waits(self, inst)
    return _orig_commit(self, inst, lazy_reg_writes=lazy_reg_writes)


def _drain_and_barrier_split(self, tick_clock, wait_clock):
    drain_inst = self.nc.sync.drain()
    wait_clock.add_sem_waits(
        drain_inst.ins, ScopedClock({None: tick_clock.global_clock})
    )
    si = drain_inst.ins.sync_info
    if si is not None and si.on_wait and len(si.on_wait) > 1:
        waits = list(si.on_wait)
        drain_inst.ins.sync_info = mybir.SyncInfo(
            on_wait=[waits[0]], on_update=list(si.on_update)
        )
        for w in waits[1:]:
            nop = self.nc.sync.nop(nofuse=True)
            nop.ins.sync_info = mybir.SyncInfo(on_wait=[w], on_update=[])

    self.nc.all_engine_barrier()
    assert self.sems is not None
    popped = self.nc._tile_sem_poison_stack.pop()
    assert popped is self._sem_poison
    self.nc.clear_and_free_semaphores(list(self.sems.allocated().values()))
    self.nc.all_engine_barrier()


def _install_tile_workarounds():
    tile.TileContext._commit_instruction = _commit_instruction_split
    tile.TileContext._drain_and_barrier = _drain_and_barrier_split


def build_program(nb=NB):
    _install_tile_workarounds()
    npairs = nb // 2                           # 512
    nhalves = npairs // HALF_PAIRS             # 8
    nchunks_half = HALF_PAIRS // CHUNK_PAIRS   # 4

    nc = bass.Bass("TRN2", target_bir_lowering=False, debug=False,
                   num_devices=NCORES)
    x = nc.dram_tensor("x", [nb, N, D], F32, kind="ExternalInput").ap()
    ident = nc.dram_tensor("ident", [128, 128], F16, kind="ExternalInput").ap()
    y = nc.dram_tensor("y", [nb, TRI], F32, kind="ExternalOutput").ap()
    xflat = x.rearrange("b n d -> (b n) d")

    with tile.TileContext(nc) as tc:
        with (
            tc.tile_pool(name="const", bufs=1) as constp,
            tc.tile_pool(name="xin", bufs=16) as xinp,
            tc.tile_pool(name="xt", bufs=4) as xtp_sb,
            tc.tile_pool(name="zsb", bufs=2) as zsbp,
            tc.tile_pool(name="tgi", bufs=2) as tgip,
            tc.tile_pool(name="osb", bufs=2) as osbp,
            tc.tile_pool(name="xtps", bufs=2, space="PSUM") as xtps,
            tc.tile_pool(name="zps", bufs=2, space="PSUM") as zps,
            tc.tile_pool(name="ctps", bufs=2, space="PSUM") as ctps,
        ):
            ident_sb = constp.tile([128, 128], F16)
            nc.sync.dma_start(ident_sb[:], ident[:])

            st_state = {}     # st -> (t_gi, out_sb)
            half_bufs = {}    # h -> [xin tiles]

            def emit_loads(h):
                pair0 = h * HALF_PAIRS
                bufs = []
                for c in range(nchunks_half):
                    xb = xinp.tile([128, CHUNK_PAIRS * 256], F16, tag="xin")
                    row0 = (pair0 + c * CHUNK_PAIRS) * 128
                    src = xflat[row0:row0 + CHUNK_PAIRS * 128, :].rearrange(
                        "(l p) d -> p l d", p=128)
                    nc.gpsimd.dma_start(
                        xb[:].rearrange("p (l d) -> p l d", d=256), src)
                    bufs.append(xb)
                half_bufs[h] = bufs

            def emit_transposes(xbufs, gg, sub):
                """Transpose 4 pairs (sub in {0,1} of 8-pair group gg)."""
                xtp = xtps.tile([128, 1024], F16, tag="xtps")
                p0 = gg * 8 + sub * 4          # first pair in half
                cidx, l0 = divmod(p0, CHUNK_PAIRS)
                for pl in range(4):
                    lc = l0 + pl
                    for k in range(2):
                        nc.tensor.transpose(
                            xtp[:, pl * 256 + k * 128:pl * 256 + (k + 1) * 128],
                            xbufs[cidx][:, lc * 256 + k * 128:lc * 256 + (k + 1) * 128],
                            ident_sb[:])
                xt = xtp_sb.tile([128, 1024], F16, tag="xt")
                nc.vector.tensor_copy(xt[:], xtp[:])
                return xt

            def emit_mms(xts, zp):
                for p in range(8):
                    xt = xts[p // 4]
                    q = (p % 4) * 256
                    zslot = zp[:, p * 128:(p + 1) * 128]
                    nc.tensor.matmul(zslot, xt[:, q:q + 128], xt[:, q:q + 128],
                                     start=True, stop=False,
                                     skip_group_check=True)
                    nc.tensor.matmul(zslot, xt[:, q + 128:q + 256],
                                     xt[:, q + 128:q + 256],
                                     start=False, stop=True,
                                     skip_group_check=True)

            def emit_z_evac(zp, z_sb, gg):
                # selective: drop cross-batch junk; z_sb [p=(g,i), (l, j)]
                zr = zp[:].rearrange("p (l c) -> p l c", c=128)
                zd = z_sb[:].rearrange("p (l j) -> p l j", j=64)
                src0 = zr[0:64, :, 0:64]
                dst0 = zd[0:64, gg * 8:(gg + 1) * 8, :]
                src1 = zr[64:128, :, 64:128]
                dst1 = zd[64:128, gg * 8:(gg + 1) * 8, :]
                if gg % 4 == 0:
                    nc.vector.tensor_copy(dst0, src0)
                    nc.vector.tensor_copy(dst1, src1)
                else:
                    nc.scalar.copy(dst0, src0)
                    nc.scalar.copy(dst1, src1)

            def make_stagec_steps(h, z_sb):
                """Stage C for half h as 8 closures, interleaved into the
                next half's matmul phase so PE never stalls on ct evacs.
                z_sb [p=(g,i), (l, j)] -> t_gi [p=l(+hb), (j, g, i)]"""
                st, half = divmod(h, 2)
                hb = half * 64
                t_gi, _ = st_state[st]
                zr = z_sb[:].rearrange("p (l j) -> p j l", j=64)

                def step(j8):
                    ct = ctps.tile([128, 1024], F16, tag="ctps",
                                   name=f"ct{h}_{j8}")
                    for jj in range(8):
                        nc.tensor.transpose(
                            ct[hb:hb + 64, jj * 128:(jj + 1) * 128],
                            zr[:, j8 * 8 + jj, :], ident_sb[:])
                    dst = t_gi[hb:hb + 64, j8 * 1024:(j8 + 1) * 1024]
                    if j8 % 2 == 0:
                        nc.vector.tensor_copy(dst, ct[hb:hb + 64, :])
                    else:
                        nc.scalar.copy(dst, ct[hb:hb + 64, :])

                return [lambda j8=j8: step(j8) for j8 in range(8)]

            def emit_zphase(h, steps):
                st, half = divmod(h, 2)
                if half == 0:
                    st_state[st] = (
                        tgip.tile([128, 64 * 128], F16, tag="tgi", name=f"tgi{st}"),
                        osbp.tile([128, 2 * TRI], F32, tag="osb", name=f"osb{st}"),
                    )
                xbufs = half_bufs.pop(h)
                z_sb = zsbp.tile([128, HALF_PAIRS * 64], F16, tag="zsb")
                prev = None
                for gg in range(8):
                    xts = (emit_transposes(xbufs, gg, 0),
                           emit_transposes(xbufs, gg, 1))
                    if prev is not None:
                        emit_mms(prev[0], prev[1])
                        emit_z_evac(prev[1], z_sb, prev[2])
                    if steps is not None:
                        steps[gg]()
                    zp = zps.tile([128, 1024], F32, tag="zps")
                    prev = (xts, zp, gg)
                emit_mms(prev[0], prev[1])
                emit_z_evac(prev[1], z_sb, prev[2])
                return z_sb

            def emit_comp(h, engines):
                """Triangle compaction copies for half h, subset by engine."""
                st, half = divmod(h, 2)
                hb = half * 64
                t_gi, out_sb = st_state[st]
                tsrc = t_gi[hb:hb + 64, :].rearrange(
                    "p (j g i) -> p g i j", g=2, i=64)
                odst = out_sb[hb:hb + 64, :].rearrange("p (g t) -> p g t", g=2)
                # halves emitted at/after the last load issue must not use
                # gpsimd: a queue drain there blocks until all loads land
                gp_ok = h < nhalves - 1
                for i in range(1, 64):
                    if h == nhalves - 1:
                        eng = "act" if i % 4 == 1 else "dve"
                    else:
                        eng = ("gp", "act", "dve")[i % 3]
                        if not gp_ok and eng == "gp":
                            eng = ("dve", "act")[i % 2]
                    if eng not in engines:
                        continue
                    off = i * (i - 1) // 2
                    src = tsrc[:, :, i, 0:i]
                    dst = odst[:, :, off:off + i]
                    if eng == "dve":
                        nc.vector.tensor_copy(dst, src)
                    elif eng == "act":
                        nc.scalar.copy(dst, src)
                    else:
                        nc.gpsimd.tensor_copy(dst, src)

            def emit_store_st(st):
                _, out_sb = st_state[st]
                base = st * 2 * ST_PAIRS
                ydst = y[base: base + 2 * ST_PAIRS, :].rearrange(
                    "(l g) t -> l (g t)", g=2)
                nc.sync.dma_start(ydst, out_sb[:])

            def emit_store_half(h):
                st, half = divmod(h, 2)
                hb = half * 64
                _, out_sb = st_state[st]
                base = st * 2 * ST_PAIRS + half * 2 * HALF_PAIRS
                ydst = y[base: base + 2 * HALF_PAIRS, :].rearrange(
                    "(l g) t -> l (g t)", g=2)
                nc.sync.dma_start(ydst, out_sb[hb:hb + 64, :])

            # ---- software-pipelined emission over halves ----------------
            # Tile orders hazards by emission order, so a half's store must
            # be emitted after ALL its compaction shares (incl. deferred
            # gpsimd ones).  Schedule at loop index hh:
            #   loads(hh); zphase(hh-1) w/ interleaved stageC(hh-2);
            #   comps(hh-2) on DVE/ACT; gp comps + store for half hh-3.
            z_sbs = {}
            for hh in range(nhalves):
                emit_loads(hh)
                if hh >= 1:
                    k = hh - 1
                    steps = (make_stagec_steps(k - 1, z_sbs[k - 1])
                             if k >= 1 else None)
                    z_sbs[k] = emit_zphase(k, steps)
                    if k >= 1:
                        emit_comp(k - 1, engines=("dve", "act"))
                m = hh - 3
                if 0 <= m < nhalves - 3:
                    emit_comp(m, engines=("gp",))
                    emit_store_half(m)
            # flush: half 5's store (its comps were emitted at hh=7);
            # zphase(7) w/ stageC(6); comps(6); store(6); stageC(7); ...
            emit_comp(nhalves - 3, engines=("gp",))
            emit_store_half(nhalves - 3)
            k = nhalves - 1
            z_sbs[k] = emit_zphase(k, make_stagec_steps(k - 1, z_sbs[k - 1]))
            emit_comp(k - 1, engines=("dve", "act"))
            emit_comp(k - 1, engines=("gp",))
            emit_store_half(k - 1)
            for s in make_stagec_steps(k, z_sbs[k]):
                s()
            emit_comp(k, engines=("dve", "act"))
            emit_store_half(k)
    return nc


_PROGRAM_CACHE = {}


def _get_program():
    if "nc" not in _PROGRAM_CACHE:
        _PROGRAM_CACHE["nc"] = build_program()
    return _PROGRAM_CACHE["nc"]


def kernel(inputs):
    from concourse.bass_utils import run_bass_kernel_spmd

    x = np.asarray(inputs, dtype=np.float32)
    assert x.shape == (B, N, D), x.shape
    nc = _get_program()
    eye = np.eye(128, dtype=np.float16)
    in_maps = [
        {"x": np.ascontiguousarray(x[i * NB:(i + 1) * NB]), "ident": eye}
        for i in range(NCORES)
    ]
    res = run_bass_kernel_spmd(nc, in_maps, list(range(NCORES)))
    out = np.concatenate([res.results[i]["y"] for i in range(NCORES)], axis=0)
    return out.astype(np.float32, copy=False)
